# revision 53
# baseline (speedup 1.0000x reference)
"""Low-rank bilinear attention kernel for Trainium2 (Bass/Tile), 8 NeuronCores.

Math: alpha[b,l,p] = sum_c wt_c * (sum_a tanh(p1[b,p,a]*p2[b,l,a]) * Wh[c,a] + bh_c) + bt
    = sum_a v_a * tanh(p1[b,p,a]*p2[b,l,a]) + const
  with v = wt @ Wh (weight fold), const = wt @ bh + bt (added on host).
  p1 = x1 @ W1.T, p2 = x2 @ W2.T.

Sharding: data-parallel over B (8 batches -> 8 cores). Weights replicated.

Device layout per core: A (1024) split into 8 blocks of 128 on partitions;
loop j (A-block) outer, label-group inner. The free dim of the main tiles
is sub-folded (p,l)-minor: column index s*(4*196) + p*4 + q holds label
l = s*4+q. Every DVE operand keeps innermost AP stride +-1, which is what
the DVE 2x perf mode requires:
  p1rep[:, p*4+q] = proj_psum[:, p]   (one cheap DVE broadcast copy per j)
  m = p1rep * p2T[j]-slice    -- in1 AP [[0,P],[1,4]], bf16, 2x mode
  h = tanh(m) on ACT -- the roofline engine (16 instrs of [128,7840];
      two label groups share one m/h tile to halve ACT instr overhead)
  contraction over A: bf16 matmuls with zero-padded v strips (lhsT [128,8],
  only column c nonzero) accumulating into 4 live [8,490] PSUM tiles
  (j=0 starts, j=7 stops); one DVE copy + DMA per group at the end.
The j=0 projection runs in two P-halves and the first tiles' tanh is
split so the ACT pipeline fills early. Host post-pass adds const and
un-permutes the sub-folded output.
"""

import os
import sys

import numpy as np

if "/opt/trn_rl_repo" not in sys.path:
    sys.path.insert(0, "/opt/trn_rl_repo")

import concourse.bass as bass
from concourse import bacc
import concourse.mybir as mybir
from concourse.bass import AP
from concourse.bass_utils import run_bass_kernel_spmd
from concourse.tile import TileContext

B, P, L = 8, 196, 80
D1, D2, A = 2048, 300, 1024
NBLK = A // 128          # 8 A-blocks
ND1 = D1 // 128          # 16 d-chunks for W1
D2P = 384                # D2 padded to 3*128
ND2 = D2P // 128         # 3
G = 20                   # labels per group
NG = L // G              # 4 groups
GW = G * P               # 3920 free width of one group tile
NCH = 8                  # contraction chunks per group
CW = GW // NCH           # 490 columns per chunk (fits one PSUM bank)
VS = 15                  # width of one zero-padded v strip
SUB = 4                  # l-minor fold: column index s*(SUB*P) + p*SUB + q
NSUB = G // SUB          # 5 sub-blocks per group tile

F32 = mybir.dt.float32
BF16 = mybir.dt.bfloat16
TANH = mybir.ActivationFunctionType.Tanh

_LAST_PERF = {}


def _build():
    nc = bacc.Bacc(None, target_bir_lowering=False)

    x1_d = nc.declare_dram_parameter("x1t", [128, ND1 * P], BF16,
                                     isOutput=False)
    w1_d = nc.declare_dram_parameter("w1r", [A, D1], BF16, isOutput=False)
    x2_d = nc.declare_dram_parameter("x2t", [128, ND2 * L], F32,
                                     isOutput=False)
    w2_d = nc.declare_dram_parameter("w2r", [A, D2P], F32, isOutput=False)
    vz_d = nc.declare_dram_parameter("vzd", [128, NBLK * VS], BF16,
                                     isOutput=False)
    out_d = nc.declare_dram_parameter("alpha", [NG * GW], F32, isOutput=True)

    with TileContext(nc) as tc:
        with (
            tc.tile_pool(name="const", bufs=1) as cpool,
            tc.tile_pool(name="w1", bufs=3) as w1p,
            tc.tile_pool(name="w2", bufs=2) as w2p,
            tc.tile_pool(name="p1rep", bufs=2) as rp,
            tc.tile_pool(name="m", bufs=4) as mp,
            tc.tile_pool(name="tanh", bufs=4) as hp,
            tc.tile_pool(name="alphas", bufs=2) as alp,
        ):
            # Warm the ACT tanh table first thing.
            warm = cpool.tile([1, 2], F32)
            nc.vector.memset(warm[:, :], 0.0)
            nc.scalar.activation(warm[:, :], warm[:, :], TANH)

            # Inputs. x1T arrives in four slices so the first projection
            # matmuls can start as soon as the early k-chunks land.
            x1T = cpool.tile([128, ND1 * P], BF16)
            w1_sb0 = w1p.tile([128, D1], BF16, tag="w1", name="w1sb_0")
            nc.sync.dma_start(out=w1_sb0[:, :D1 // 2], in_=w1_d[0:128, :D1 // 2])
            Q = ND1 * P // 4
            # Issue the x1 head slice from gpsimd so it doesn't queue
            # behind the w1 issue on the sync engine.
            nc.gpsimd.dma_start(out=x1T[:, :Q], in_=x1_d[:, :Q])
            nc.sync.dma_start(out=w1_sb0[:, D1 // 2:], in_=w1_d[0:128, D1 // 2:])
            x2T = cpool.tile([128, ND2 * L], F32)
            vzr = cpool.tile([128, NBLK * VS], BF16)

            p2T = cpool.tile([128, NBLK * L], BF16)

            with (
                tc.tile_pool(name="ps_p2", bufs=1, space="PSUM") as ps2,
                tc.tile_pool(name="ps_p1", bufs=2, space="PSUM") as ps1,
                tc.tile_pool(name="ps_al", bufs=1, space="PSUM") as psa,
            ):
                def p2blk(j):
                    w2_sb = w2p.tile([128, D2P], F32, tag="w2",
                                     name=f"w2sb_{j}")
                    nc.sync.dma_start(out=w2_sb[:, :],
                                      in_=w2_d[j * 128:(j + 1) * 128, :])
                    pm = ps2.tile([128, L], F32, tag="p2ps",
                                  name=f"p2ps_{j}")
                    for kk in range(ND2):
                        nc.tensor.matmul(pm[:, :],
                                         lhsT=w2_sb[:, kk * 128:(kk + 1) * 128],
                                         rhs=x2T[:, kk * L:(kk + 1) * L],
                                         start=(kk == 0), stop=(kk == ND2 - 1))
                    nc.vector.tensor_copy(p2T[:, j * L:(j + 1) * L], pm[:, :])

                def proj(j, w1_sb=None):
                    if w1_sb is None:
                        w1_sb = w1p.tile([128, D1], BF16, tag="w1",
                                         name=f"w1sb_{j}")
                        nc.sync.dma_start(out=w1_sb[:, :],
                                          in_=w1_d[j * 128:(j + 1) * 128, :])
                    pm1 = ps1.tile([128, P], F32, tag="p1ps",
                                   name=f"p1ps_{j}")
                    for k in range(ND1):
                        nc.tensor.matmul(pm1[:, :],
                                         lhsT=w1_sb[:, k * 128:(k + 1) * 128],
                                         rhs=x1T[:, k * P:(k + 1) * P],
                                         start=(k == 0), stop=(k == ND1 - 1))
                    return pm1

                nc.scalar.dma_start(out=x2T[:, :], in_=x2_d[:, :])
                p2blk(0)
                for q in range(1, 4):
                    nc.sync.dma_start(out=x1T[:, q * Q:(q + 1) * Q],
                                      in_=x1_d[:, q * Q:(q + 1) * Q])
                nc.sync.dma_start(out=vzr[:, :], in_=vz_d[:, :])
                # j=0 projection in two P-halves so the first tile's DVE/ACT
                # work can start when the first half-chain completes.
                PA = P // 2
                pmA = ps1.tile([128, PA], F32, tag="p1ps", name="p1psA")
                for k in range(ND1):
                    nc.tensor.matmul(pmA[:, :],
                                     lhsT=w1_sb0[:, k * 128:(k + 1) * 128],
                                     rhs=x1T[:, k * P:k * P + PA],
                                     start=(k == 0), stop=(k == ND1 - 1))
                pmB = ps1.tile([128, P - PA], F32, tag="p1ps", name="p1psB")
                for k in range(ND1):
                    nc.tensor.matmul(pmB[:, :],
                                     lhsT=w1_sb0[:, k * 128:(k + 1) * 128],
                                     rhs=x1T[:, k * P + PA:(k + 1) * P],
                                     start=(k == 0), stop=(k == ND1 - 1))
                proj_ps = {}
                proj_ps[1] = proj(1)

                al_ps = [psa.tile([NCH, CW], F32, tag=f"al{g}",
                                  name=f"alps_{g}")
                         for g in range(NG)]

                def half_cast(pm, p1rep, p0, pn):
                    a = pm[:, :]
                    rin = AP(a.tensor, a.offset,
                             [a.ap[0], [1, pn], [0, SUB]])
                    ao = p1rep[:, p0 * SUB:(p0 + pn) * SUB]
                    rout = AP(ao.tensor, ao.offset,
                              [ao.ap[0], [SUB, pn], [1, SUB]])
                    nc.vector.tensor_copy(rout, rin)

                def half_tts(m, c0, p1rep, j, g, p0, pn):
                    for s in range(NSUB):
                        x = p2T[:, j * L + g * G + s * SUB:
                                j * L + g * G + (s + 1) * SUB]
                        in1 = AP(x.tensor, x.offset,
                                 [x.ap[0], [0, pn], [1, SUB]])
                        nc.vector.tensor_tensor(
                            m[:, c0 + s * SUB * P + p0 * SUB:
                              c0 + s * SUB * P + (p0 + pn) * SUB],
                            p1rep[:, p0 * SUB:(p0 + pn) * SUB], in1,
                            mybir.AluOpType.mult)

                for j in range(NBLK):
                    # p1rep[:, p*SUB+q] = proj_psum[:, p], bf16 out.
                    # Only SUB-fold replication: keeps the broadcast copy
                    # cheap (1x mode) while the TTs still hit 2x mode.
                    p1rep = rp.tile([128, SUB * P], BF16, tag="p1rep")
                    if j == 0:
                        half_cast(pmA, p1rep, 0, PA)
                    else:
                        pm1 = proj_ps.pop(j)
                        a = pm1[:, :]
                        rin = AP(a.tensor, a.offset,
                                 [a.ap[0], [1, P], [0, SUB]])
                        ao = p1rep[:, :]
                        rout = AP(ao.tensor, ao.offset,
                                  [ao.ap[0], [SUB, P], [1, SUB]])
                        nc.vector.tensor_copy(rout, rin)
                    if j + 2 < NBLK:
                        proj_ps[j + 2] = proj(j + 2)

                    for gp in range(NG // 2):
                        # Two label groups share one m/h tile so ACT runs
                        # half as many (double-size) tanh instructions.
                        m = mp.tile([128, 2 * GW], BF16, tag="m")
                        for gh in range(2):
                            g = 2 * gp + gh
                            if j == 0 and g == 0:
                                half_tts(m, 0, p1rep, 0, 0, 0, PA)
                                half_cast(pmB, p1rep, PA, P - PA)
                                half_tts(m, 0, p1rep, 0, 0, PA, P - PA)
                            else:
                                half_tts(m, gh * GW, p1rep, j, g, 0, P)
                        h = hp.tile([128, 2 * GW], BF16, tag="h")
                        if j == 0 and gp == 0:
                            # Ramp: tanh g0 in P-halves, then g1.
                            for p0, pn in ((0, PA), (PA, P - PA)):
                                ai = m[:, :]
                                hin = AP(ai.tensor, ai.offset + p0 * SUB,
                                         [ai.ap[0], [SUB * P, NSUB],
                                          [1, pn * SUB]])
                                aoh = h[:, :]
                                hout = AP(aoh.tensor, aoh.offset + p0 * SUB,
                                          [aoh.ap[0], [SUB * P, NSUB],
                                           [1, pn * SUB]])
                                nc.scalar.activation(hout, hin, TANH)
                            nc.scalar.activation(h[:, GW:], m[:, GW:], TANH)
                        elif j <= 1 or (j == NBLK - 1 and gp == NG // 2 - 1):
                            # Ramp (DVE still catching up) and tail: split
                            # per group so ACT starts / PE drains earlier.
                            nc.scalar.activation(h[:, :GW], m[:, :GW], TANH)
                            nc.scalar.activation(h[:, GW:], m[:, GW:], TANH)
                        else:
                            nc.scalar.activation(h[:, :], m[:, :], TANH)
                        for c in range(2 * NCH):
                            g = 2 * gp + c // NCH
                            cc = c % NCH
                            nc.tensor.matmul(
                                al_ps[g][:, :],
                                lhsT=vzr[:, j * VS + 7 - cc:j * VS + VS - cc],
                                rhs=h[:, c * CW:(c + 1) * CW],
                                start=(j == 0 and cc == 0),
                                stop=(j == NBLK - 1 and cc == NCH - 1))
                        if j == NBLK - 1:
                            for gh in range(2):
                                g = 2 * gp + gh
                                alpha_sb = alp.tile([NCH, CW], F32,
                                                    tag="alpha",
                                                    name=f"alpha_{g}")
                                nc.vector.tensor_copy(alpha_sb[:, :],
                                                      al_ps[g][:, :])
                                nc.sync.dma_start(
                                    out=out_d[g * GW:(g + 1) * GW],
                                    in_=alpha_sb[:, :])
                    if j + 1 < NBLK:
                        p2blk(j + 1)
    nc.finalize()
    return nc


def _install_axon_trace_hook() -> bool:
    """Install the NTFF profiling hook for axon runs (test-time only)."""
    try:
        import contextlib
        import ctypes
        import types

        so_path = "/opt/axon/libaxon_pjrt.so"
        if not os.path.exists(so_path):
            return False
        lib = ctypes.CDLL(so_path)
        if not hasattr(lib, "axon_start_nrt_profile"):
            return False
        lib.axon_start_nrt_profile.argtypes = [
            ctypes.POINTER(ctypes.c_int64), ctypes.c_size_t]
        lib.axon_start_nrt_profile.restype = ctypes.c_int64
        lib.axon_stop_nrt_profile.argtypes = [ctypes.c_char_p]
        lib.axon_stop_nrt_profile.restype = ctypes.c_int64

        @contextlib.contextmanager
        def _hook(output_dir, device_ids):
            import jax
            jax.devices()
            if device_ids:
                ids = (ctypes.c_int64 * len(device_ids))(*device_ids)
                rc = lib.axon_start_nrt_profile(ids, len(device_ids))
            else:
                rc = lib.axon_start_nrt_profile(None, 0)
            if rc != 0:
                raise RuntimeError(f"axon_start_nrt_profile rc={rc}")
            try:
                yield
            finally:
                n = lib.axon_stop_nrt_profile(str(output_dir).encode())
                print(f"profile: {n} file(s) written to {output_dir}",
                      file=sys.stderr)

        mod = types.ModuleType("antenv.axon_hooks")
        mod.get_axon_ntff_profile_hook = lambda: _hook
        mod.set_axon_ntff_profile_hook = lambda h: None
        sys.modules["antenv.axon_hooks"] = mod

        import concourse.bass_utils as bu
        bu.upload_artifacts = lambda tmpdir: f"local://{tmpdir}"
        return True
    except Exception as e:  # pragma: no cover
        print(f"trace hook install failed: {e}", file=sys.stderr)
        return False


def kernel(x1, x2, W1, W2, Wh, bh, wt, bt):
    import ml_dtypes

    x1 = np.ascontiguousarray(np.asarray(x1, dtype=np.float32))
    x2 = np.ascontiguousarray(np.asarray(x2, dtype=np.float32))
    W1 = np.asarray(W1, dtype=np.float32)
    W2 = np.asarray(W2, dtype=np.float32)
    Wh = np.asarray(Wh, dtype=np.float32)
    bh = np.asarray(bh, dtype=np.float32)
    wt = np.asarray(wt, dtype=np.float32)
    bt = np.float32(np.asarray(bt))

    # Weight folding (host): rank-1 output head collapses into v.
    v = wt @ Wh                                   # [A]
    const_val = np.float32(wt @ bh + np.float32(bt))

    # w1 blocks: w1r[j*128+d, k*128+a] = W1[j*128+a, k*128+d]  (bf16)
    w1r = np.ascontiguousarray(
        W1.reshape(NBLK, 128, ND1, 128).transpose(0, 3, 2, 1).reshape(A, D1)
        .astype(ml_dtypes.bfloat16))
    # w2 blocks: w2r[j*128+d, kk*128+a] = W2[j*128+a, kk*128+d]  (fp32, padded)
    w2tp = np.zeros((D2P, A), dtype=np.float32)
    w2tp[:D2] = W2.T
    w2r = np.ascontiguousarray(
        w2tp.reshape(ND2, 128, NBLK, 128).transpose(2, 1, 0, 3).reshape(A, D2P))
    # zero-padded v strips: vzd[:, j*VS+7] = v block j, else 0.
    vzd = np.zeros((128, NBLK * VS), dtype=np.float32)
    for j in range(NBLK):
        vzd[:, j * VS + 7] = v[j * 128:(j + 1) * 128]
    vzd = np.ascontiguousarray(vzd.astype(ml_dtypes.bfloat16))

    nc = _build()

    # Host pre-transposes (layout only).
    x2tp = np.zeros((D2P, L), dtype=np.float32)
    in_maps = []
    for b in range(B):
        x1t = np.ascontiguousarray(
            x1[b].T.reshape(ND1, 128, P).transpose(1, 0, 2).reshape(128, -1)
            .astype(ml_dtypes.bfloat16))
        x2tp[:D2] = x2[b].T
        x2t = np.ascontiguousarray(
            x2tp.reshape(ND2, 128, L).transpose(1, 0, 2).reshape(128, -1))
        in_maps.append({
            "x1t": x1t,
            "x2t": x2t,
            "w1r": w1r,
            "w2r": w2r,
            "vzd": vzd,
        })

    trace = os.environ.get("KERNEL_TRACE", "0") == "1"
    if trace:
        trace = _install_axon_trace_hook()
    res = run_bass_kernel_spmd(nc, in_maps, list(range(B)), trace=trace,
                               tmpdir=os.environ.get("KERNEL_TMPDIR") or None)
    _LAST_PERF.clear()
    _LAST_PERF["exec_time_ns"] = res.exec_time_ns
    _LAST_PERF["profile_json"] = res.profile_json

    # Un-permute: group g's tile column s*(SUB*P) + p*SUB + q holds
    # (label g*G + s*SUB + q, position p).
    out = np.empty((B, L, P), dtype=np.float32)
    for b in range(B):
        flat = res.results[b]["alpha"]
        for g in range(NG):
            blk = flat[g * GW:(g + 1) * GW].reshape(NSUB, P, SUB)
            out[b, g * G:(g + 1) * G, :] = \
                blk.transpose(0, 2, 1).reshape(G, P)
    out += const_val
    return out
